# revision 4
# baseline (speedup 1.0000x reference)
"""Trainium2 Bass kernel for the per-game CriticNetwork (MoE-routed MLP).

Network (per sample b, with game g = idx[b]):
    h1  = relu(W1[g] @ state[b] + b1[g])          # [600]
    h2  = W2s @ h1 + b2s + W2a[g] @ action[b]     # [500]
    q   = W3[g] . relu(h2) + b3[g]                # scalar

Strategy: all MoE routing happens on the HOST. idx is (stably) sorted into
per-game contiguous segments, each segment is padded up to 512-sample tiles,
and the tile list is padded to a fixed 72 tiles (9 per core x 8 cores).
Every tile is single-game, so the device kernel is a fully static dense
pipeline; the host pre-gathers per-tile weight views (pre-transposed for the
PE's lhsT layout) so the device does zero routing and zero transposes.

Device per tile t (512 samples, fp32 everywhere, PSUM accumulation):
    L1: 5 matmuls  [K=128(d), M=128(h1 chunk), N=512(b)] + fused relu+bias (ACT)
    L2: 4 m-chunks x (5 K=128 matmuls of shared W2s + 1 K=16 matmul of W2a[g])
        accumulated in PSUM + fused relu+bias (ACT)
    L3: 4 accumulating matmuls [K=128, M=1, N=512] with W3[g] -> q[1, 512]
b3 is added on the host after gathering.
"""

import numpy as np

import concourse.bass as bass
import concourse.mybir as mybir
import concourse.tile as tile
from concourse import bacc
from concourse.bass import ts
from concourse.bass_utils import run_bass_kernel_spmd

F32 = mybir.dt.float32
RELU = mybir.ActivationFunctionType.Relu

G = 8          # games
D = 128        # state dim
A = 16         # action dim
H1 = 600       # hidden 1 (padded to 640 = 5 * 128)
H2 = 500       # hidden 2 (padded to 512 = 4 * 128)
B = 32768      # batch
H1P, H2P = 640, 512
K1 = H1P // 128   # 5 h1 chunks
M2 = H2P // 128   # 4 h2 chunks
T = 512        # samples per tile (one PSUM bank of fp32)
NCORES = 8
NT = 9         # tiles per core; 72 total >= 64 + 7 worst-case segment padding
BPC = NT * T   # 4608 lanes per core

_NC = None


def build_nc():
    nc = bacc.Bacc("TRN2", target_bir_lowering=False, debug=False,
                   num_devices=NCORES)

    stateT = nc.declare_dram_parameter("stateT", [D, BPC], F32, isOutput=False)
    aT = nc.declare_dram_parameter("aT", [A, BPC], F32, isOutput=False)
    w1t = nc.declare_dram_parameter("w1t", [NT, D, H1P], F32, isOutput=False)
    b1t = nc.declare_dram_parameter("b1t", [NT, 128, K1], F32, isOutput=False)
    w2st = nc.declare_dram_parameter("w2st", [H1P, H2P], F32, isOutput=False)
    w2at = nc.declare_dram_parameter("w2at", [NT, A, H2P], F32, isOutput=False)
    b2st = nc.declare_dram_parameter("b2st", [128, M2], F32, isOutput=False)
    w3t = nc.declare_dram_parameter("w3t", [NT, 128, M2], F32, isOutput=False)
    q = nc.declare_dram_parameter("q", [1, BPC], F32, isOutput=True)

    with tile.TileContext(nc) as tc:
        with (
            tc.tile_pool(name="const", bufs=1) as const,
            tc.tile_pool(name="wts", bufs=2) as wts,
            tc.tile_pool(name="acts", bufs=2) as acts,
            tc.tile_pool(name="outp", bufs=2) as outp,
            tc.tile_pool(name="ps1", bufs=2, space="PSUM") as ps1p,
            tc.tile_pool(name="ps2", bufs=2, space="PSUM") as ps2p,
            tc.tile_pool(name="psq", bufs=2, space="PSUM") as psqp,
        ):
            # Shared weights, loaded once. w2st_sb[p, c, j] = W2sT[c*128+p, j]
            w2st_sb = const.tile([128, K1, H2P], F32)
            nc.sync.dma_start(
                w2st_sb[:], w2st.ap().rearrange("(c p) n -> p c n", p=128))
            b2st_sb = const.tile([128, M2], F32)
            nc.sync.dma_start(b2st_sb[:], b2st.ap())

            for t in range(NT):
                # Per-tile weight views (host pre-gathered by game id).
                w1t_sb = wts.tile([D, H1P], F32, tag="w1")
                nc.sync.dma_start(w1t_sb[:], w1t[t])
                b1t_sb = wts.tile([128, K1], F32, tag="b1")
                nc.sync.dma_start(b1t_sb[:], b1t[t])
                w2at_sb = wts.tile([A, H2P], F32, tag="w2a")
                nc.sync.dma_start(w2at_sb[:], w2at[t])
                w3t_sb = wts.tile([128, M2], F32, tag="w3")
                nc.sync.dma_start(w3t_sb[:], w3t[t])

                st_sb = acts.tile([D, T], F32, tag="st")
                nc.sync.dma_start(st_sb[:], stateT[:, ts(t, T)])
                at_sb = acts.tile([A, T], F32, tag="at")
                nc.sync.dma_start(at_sb[:], aT[:, ts(t, T)])

                # L1: h1 = relu(W1[g] @ state + b1[g]), chunked over H1P.
                h1 = acts.tile([128, K1, T], F32, tag="h1")
                for c in range(K1):
                    ps1 = ps1p.tile([128, T], F32, tag="ps1")
                    nc.tensor.matmul(ps1[:], w1t_sb[:, ts(c, 128)], st_sb[:],
                                     start=True, stop=True)
                    nc.scalar.activation(h1[:, c, :], ps1[:], RELU,
                                         bias=b1t_sb[:, c:c + 1])

                # L2: h2 = W2s @ h1 + W2a[g] @ action + b2s, relu.
                hf = acts.tile([128, M2, T], F32, tag="hf")
                for m in range(M2):
                    ps2 = ps2p.tile([128, T], F32, tag="ps2")
                    for c in range(K1):
                        nc.tensor.matmul(ps2[:], w2st_sb[:, c, ts(m, 128)],
                                         h1[:, c, :], start=(c == 0),
                                         stop=False)
                    nc.tensor.matmul(ps2[:], w2at_sb[:, ts(m, 128)], at_sb[:],
                                     start=False, stop=True)
                    nc.scalar.activation(hf[:, m, :], ps2[:], RELU,
                                         bias=b2st_sb[:, m:m + 1])

                # L3: q = W3[g] . hf  (partition reduction via M=1 matmuls).
                psq = psqp.tile([1, T], F32, tag="psq")
                for m in range(M2):
                    nc.tensor.matmul(psq[:], w3t_sb[:, m:m + 1], hf[:, m, :],
                                     start=(m == 0), stop=(m == M2 - 1))
                q_sb = outp.tile([1, T], F32, tag="q")
                nc.scalar.activation(q_sb[:], psq[:],
                                     mybir.ActivationFunctionType.Copy)
                nc.sync.dma_start(q[0:1, ts(t, T)], q_sb[:])

    nc.compile()
    return nc


def _get_nc():
    global _NC
    if _NC is None:
        _NC = build_nc()
    return _NC


def _plan_tiles(idx):
    """Stable-sort samples by game, pad each game segment to 512-sample
    tiles, pad the tile list to the fixed 72. Returns (sel, valid, gids):
    sel[t, l] = original sample index feeding lane l of tile t."""
    perm = np.argsort(idx, kind="stable")
    counts = np.bincount(idx, minlength=G)
    ntot = NCORES * NT
    sel = np.zeros((ntot, T), np.int64)
    valid = np.zeros((ntot, T), bool)
    gids = np.zeros(ntot, np.int64)
    pos, t = 0, 0
    for g in range(G):
        cg = int(counts[g])
        for k in range((cg + T - 1) // T):
            n = min(T, cg - k * T)
            lanes = perm[pos:pos + n]
            sel[t, :n] = lanes
            valid[t, :n] = True
            if n < T:
                sel[t, n:] = lanes[0]
            gids[t] = g
            pos += n
            t += 1
    assert t <= ntot, f"tile plan overflow: {t} > {ntot}"
    return sel, valid, gids


def kernel(**inputs):
    state = np.ascontiguousarray(np.asarray(inputs["state"], np.float32))
    action = np.ascontiguousarray(np.asarray(inputs["action"], np.float32))
    idx = np.asarray(inputs["idx"]).astype(np.int64)
    W1 = np.asarray(inputs["W1"], np.float32)
    b1 = np.asarray(inputs["b1"], np.float32)
    W2s = np.asarray(inputs["W2s"], np.float32)
    b2s = np.asarray(inputs["b2s"], np.float32)
    W2a = np.asarray(inputs["W2a"], np.float32)
    W3 = np.asarray(inputs["W3"], np.float32)
    b3 = np.asarray(inputs["b3"], np.float32)
    assert state.shape == (B, D) and action.shape == (B, A)

    sel, valid, gids = _plan_tiles(idx)

    # Pre-transposed / padded weight views, indexed per tile by game id.
    W1T_all = np.zeros((G, D, H1P), np.float32)
    W1T_all[:, :, :H1] = W1.transpose(0, 2, 1)
    b1P = np.zeros((G, H1P), np.float32)
    b1P[:, :H1] = b1
    b1c_all = np.ascontiguousarray(b1P.reshape(G, K1, 128).transpose(0, 2, 1))
    W2sTP = np.zeros((H1P, H2P), np.float32)
    W2sTP[:H1, :H2] = W2s.T
    W2aT_all = np.zeros((G, A, H2P), np.float32)
    W2aT_all[:, :, :H2] = W2a.transpose(0, 2, 1)
    b2sP = np.zeros(H2P, np.float32)
    b2sP[:H2] = b2s
    b2st = np.ascontiguousarray(b2sP.reshape(M2, 128).T)
    W3P = np.zeros((G, H2P), np.float32)
    W3P[:, :H2] = W3
    W3T_all = np.ascontiguousarray(W3P.reshape(G, M2, 128).transpose(0, 2, 1))

    in_maps = []
    for c in range(NCORES):
        tsl = slice(c * NT, (c + 1) * NT)
        lanes = sel[tsl].reshape(-1)
        gt = gids[tsl]
        in_maps.append({
            "stateT": np.ascontiguousarray(state[lanes].T),
            "aT": np.ascontiguousarray(action[lanes].T),
            "w1t": np.ascontiguousarray(W1T_all[gt]),
            "b1t": np.ascontiguousarray(b1c_all[gt]),
            "w2st": W2sTP,
            "w2at": np.ascontiguousarray(W2aT_all[gt]),
            "b2st": b2st,
            "w3t": np.ascontiguousarray(W3T_all[gt]),
        })

    res = run_bass_kernel_spmd(_get_nc(), in_maps, list(range(NCORES))).results
    qv = np.concatenate([np.asarray(res[c]["q"]).reshape(-1)
                         for c in range(NCORES)])

    out = np.zeros(B, np.float32)
    flat_sel = sel.reshape(-1)
    flat_valid = valid.reshape(-1)
    out[flat_sel[flat_valid]] = qv[flat_valid]
    out += b3[idx]
    return out.astype(np.float32)


# revision 11
# speedup vs baseline: 3.4982x; 3.4982x over previous
"""Trainium2 Bass kernel for the per-game CriticNetwork (MoE-routed MLP).

Network (per sample b, with game g = idx[b]):
    h1  = relu(W1[g] @ state[b] + b1[g])          # [600]
    h2  = W2s @ h1 + b2s + W2a[g] @ action[b]     # [500]
    q   = W3[g] . relu(h2) + b3[g]                # scalar

Strategy: all MoE routing happens on the HOST. idx is (stably) sorted into
per-game contiguous segments, each segment is padded up to 512-sample tiles,
and the tile list is padded to a fixed 72 tiles (9 per core x 8 cores).
Every tile is single-game, so the device kernel is a fully static dense
pipeline; the host pre-gathers per-tile weight views (pre-transposed for the
PE's lhsT layout) so the device does zero routing and zero transposes.

Device per tile t (512 samples, fp32 everywhere, PSUM accumulation):
    L1: 5 matmuls  [K=128(d), M=128(h1 chunk), N=512(b)] + fused relu+bias (ACT)
    L2: 4 m-chunks x (5 K=128 matmuls of shared W2s + 1 K=16 matmul of W2a[g])
        accumulated in PSUM + fused relu+bias (ACT)
    L3: 4 accumulating matmuls [K=128, M=1, N=512] with W3[g] -> q[1, 512]
b3 is added on the host after gathering.
"""

import numpy as np

import concourse.bass as bass
import concourse.mybir as mybir
import concourse.tile as tile
from concourse import bacc
from concourse.bass import ts
from concourse.bass_utils import run_bass_kernel_spmd

F32 = mybir.dt.float32
RELU = mybir.ActivationFunctionType.Relu

# Matmul operand dtype: bfloat16 runs the PE at 1 cycle/row (fp32 is 4).
MM_DT = mybir.dt.bfloat16
_NP_MM_DT = mybir.dt.np(MM_DT)

G = 8          # games
D = 128        # state dim
A = 16         # action dim
H1 = 600       # hidden 1 (padded to 640 = 5 * 128)
H2 = 500       # hidden 2 (padded to 512 = 4 * 128)
B = 32768      # batch
H1P, H2P = 640, 512
K1 = H1P // 128   # 5 h1 chunks
M2 = H2P // 128   # 4 h2 chunks
T = 512        # samples per tile (one PSUM bank of fp32)
NCORES = 8
NT = 9         # tiles per core; 72 total >= 64 + 7 worst-case segment padding
BPC = NT * T   # 4608 lanes per core

_NC = None


def build_nc():
    nc = bacc.Bacc("TRN2", target_bir_lowering=False, debug=False,
                   num_devices=NCORES)

    stateT = nc.declare_dram_parameter("stateT", [D, BPC], MM_DT, isOutput=False)
    aT = nc.declare_dram_parameter("aT", [A, BPC], MM_DT, isOutput=False)
    w1t = nc.declare_dram_parameter("w1t", [NT, D, H1P], MM_DT, isOutput=False)
    b1t = nc.declare_dram_parameter("b1t", [NT, 128, K1], F32, isOutput=False)
    w2st = nc.declare_dram_parameter("w2st", [H1P, H2P], MM_DT, isOutput=False)
    w2at = nc.declare_dram_parameter("w2at", [NT, A, H2P], MM_DT, isOutput=False)
    b2st = nc.declare_dram_parameter("b2st", [128, M2], F32, isOutput=False)
    w3t = nc.declare_dram_parameter("w3t", [NT, 128, M2], MM_DT, isOutput=False)
    q = nc.declare_dram_parameter("q", [1, BPC], F32, isOutput=True)

    with tile.TileContext(nc) as tc:
        with (
            tc.tile_pool(name="const", bufs=1) as const,
            tc.tile_pool(name="wts", bufs=2) as wts,
            tc.tile_pool(name="acts", bufs=2) as acts,
            tc.tile_pool(name="outp", bufs=2) as outp,
            tc.tile_pool(name="ps1", bufs=2, space="PSUM") as ps1p,
            tc.tile_pool(name="ps2", bufs=2, space="PSUM") as ps2p,
            tc.tile_pool(name="psq", bufs=2, space="PSUM") as psqp,
        ):
            # Shared weights, loaded once. w2st_sb[p, c, j] = W2sT[c*128+p, j]
            w2st_sb = const.tile([128, K1, H2P], MM_DT)
            nc.sync.dma_start(
                w2st_sb[:], w2st.ap().rearrange("(c p) n -> p c n", p=128))
            b2st_sb = const.tile([128, M2], F32)
            nc.sync.dma_start(b2st_sb[:], b2st.ap())

            for t in range(NT):
                # Per-tile weight views (host pre-gathered by game id).
                w1t_sb = wts.tile([D, H1P], MM_DT, tag="w1")
                nc.sync.dma_start(w1t_sb[:], w1t[t])
                b1t_sb = wts.tile([128, K1], F32, tag="b1")
                nc.sync.dma_start(b1t_sb[:], b1t[t])
                w2at_sb = wts.tile([A, H2P], MM_DT, tag="w2a")
                nc.sync.dma_start(w2at_sb[:], w2at[t])
                w3t_sb = wts.tile([128, M2], MM_DT, tag="w3")
                nc.sync.dma_start(w3t_sb[:], w3t[t])

                st_sb = acts.tile([D, T], MM_DT, tag="st")
                nc.sync.dma_start(st_sb[:], stateT[:, ts(t, T)])
                at_sb = acts.tile([A, T], MM_DT, tag="at")
                nc.sync.dma_start(at_sb[:], aT[:, ts(t, T)])

                # L1: h1 = relu(W1[g] @ state + b1[g]), chunked over H1P.
                h1 = acts.tile([128, K1, T], MM_DT, tag="h1")
                for c in range(K1):
                    ps1 = ps1p.tile([128, T], F32, tag="ps1")
                    nc.tensor.matmul(ps1[:], w1t_sb[:, ts(c, 128)], st_sb[:],
                                     start=True, stop=True)
                    nc.scalar.activation(h1[:, c, :], ps1[:], RELU,
                                         bias=b1t_sb[:, c:c + 1])

                # L2: h2 = W2s @ h1 + W2a[g] @ action + b2s, relu.
                hf = acts.tile([128, M2, T], MM_DT, tag="hf")
                for m in range(M2):
                    ps2 = ps2p.tile([128, T], F32, tag="ps2")
                    for c in range(K1):
                        nc.tensor.matmul(ps2[:], w2st_sb[:, c, ts(m, 128)],
                                         h1[:, c, :], start=(c == 0),
                                         stop=False)
                    nc.tensor.matmul(ps2[:], w2at_sb[:, ts(m, 128)], at_sb[:],
                                     start=False, stop=True)
                    nc.scalar.activation(hf[:, m, :], ps2[:], RELU,
                                         bias=b2st_sb[:, m:m + 1])

                # L3: q = W3[g] . hf  (partition reduction via M=1 matmuls).
                psq = psqp.tile([1, T], F32, tag="psq")
                for m in range(M2):
                    nc.tensor.matmul(psq[:], w3t_sb[:, m:m + 1], hf[:, m, :],
                                     start=(m == 0), stop=(m == M2 - 1))
                q_sb = outp.tile([1, T], F32, tag="q")
                nc.scalar.activation(q_sb[:], psq[:],
                                     mybir.ActivationFunctionType.Copy)
                nc.sync.dma_start(q[0:1, ts(t, T)], q_sb[:])

    nc.compile()
    return nc


def _get_nc():
    global _NC
    if _NC is None:
        _NC = build_nc()
    return _NC


def _plan_tiles(idx):
    """Stable-sort samples by game, pad each game segment to 512-sample
    tiles, pad the tile list to the fixed 72. Returns (sel, valid, gids):
    sel[t, l] = original sample index feeding lane l of tile t."""
    perm = np.argsort(idx, kind="stable")
    counts = np.bincount(idx, minlength=G)
    ntot = NCORES * NT
    sel = np.zeros((ntot, T), np.int64)
    valid = np.zeros((ntot, T), bool)
    gids = np.zeros(ntot, np.int64)
    pos, t = 0, 0
    for g in range(G):
        cg = int(counts[g])
        for k in range((cg + T - 1) // T):
            n = min(T, cg - k * T)
            lanes = perm[pos:pos + n]
            sel[t, :n] = lanes
            valid[t, :n] = True
            if n < T:
                sel[t, n:] = lanes[0]
            gids[t] = g
            pos += n
            t += 1
    assert t <= ntot, f"tile plan overflow: {t} > {ntot}"
    return sel, valid, gids


def build_in_maps(inputs):
    state = np.ascontiguousarray(np.asarray(inputs["state"], np.float32))
    action = np.ascontiguousarray(np.asarray(inputs["action"], np.float32))
    idx = np.asarray(inputs["idx"]).astype(np.int64)
    W1 = np.asarray(inputs["W1"], np.float32)
    b1 = np.asarray(inputs["b1"], np.float32)
    W2s = np.asarray(inputs["W2s"], np.float32)
    b2s = np.asarray(inputs["b2s"], np.float32)
    W2a = np.asarray(inputs["W2a"], np.float32)
    W3 = np.asarray(inputs["W3"], np.float32)
    assert state.shape == (B, D) and action.shape == (B, A)

    sel, valid, gids = _plan_tiles(idx)

    # Pre-transposed / padded weight views, indexed per tile by game id.
    W1T_all = np.zeros((G, D, H1P), np.float32)
    W1T_all[:, :, :H1] = W1.transpose(0, 2, 1)
    b1P = np.zeros((G, H1P), np.float32)
    b1P[:, :H1] = b1
    b1c_all = np.ascontiguousarray(b1P.reshape(G, K1, 128).transpose(0, 2, 1))
    W2sTP = np.zeros((H1P, H2P), np.float32)
    W2sTP[:H1, :H2] = W2s.T
    W2aT_all = np.zeros((G, A, H2P), np.float32)
    W2aT_all[:, :, :H2] = W2a.transpose(0, 2, 1)
    b2sP = np.zeros(H2P, np.float32)
    b2sP[:H2] = b2s
    b2st = np.ascontiguousarray(b2sP.reshape(M2, 128).T)
    W3P = np.zeros((G, H2P), np.float32)
    W3P[:, :H2] = W3
    W3T_all = np.ascontiguousarray(W3P.reshape(G, M2, 128).transpose(0, 2, 1))

    in_maps = []
    for c in range(NCORES):
        tsl = slice(c * NT, (c + 1) * NT)
        lanes = sel[tsl].reshape(-1)
        gt = gids[tsl]
        in_maps.append({
            "stateT": np.ascontiguousarray(state[lanes].T).astype(_NP_MM_DT),
            "aT": np.ascontiguousarray(action[lanes].T).astype(_NP_MM_DT),
            "w1t": np.ascontiguousarray(W1T_all[gt]).astype(_NP_MM_DT),
            "b1t": np.ascontiguousarray(b1c_all[gt]),
            "w2st": W2sTP.astype(_NP_MM_DT),
            "w2at": np.ascontiguousarray(W2aT_all[gt]).astype(_NP_MM_DT),
            "b2st": b2st,
            "w3t": np.ascontiguousarray(W3T_all[gt]).astype(_NP_MM_DT),
        })
    return in_maps, sel, valid


def kernel(**inputs):
    idx = np.asarray(inputs["idx"]).astype(np.int64)
    b3 = np.asarray(inputs["b3"], np.float32)
    in_maps, sel, valid = build_in_maps(inputs)

    res = run_bass_kernel_spmd(_get_nc(), in_maps, list(range(NCORES))).results
    qv = np.concatenate([np.asarray(res[c]["q"]).reshape(-1)
                         for c in range(NCORES)])

    out = np.zeros(B, np.float32)
    flat_sel = sel.reshape(-1)
    flat_valid = valid.reshape(-1)
    out[flat_sel[flat_valid]] = qv[flat_valid]
    out += b3[idx]
    return out.astype(np.float32)
